# revision 7
# baseline (speedup 1.0000x reference)
"""Trainium2 Bass kernel for nn_Attention_35639638622507 (sparse_attention).

Reference computation (per batch b of 32, n=512 tokens, dim=512, 8 heads x 64):
  qkv = x @ W_qkv ; q,k,v = split
  dots = (q @ k^T) * s + skew(q @ rel^T) * s      (rel-pos bias, s = 1/8)
  out  = softmax(dots) @ v @ W_out + b_out

Strategy: data-parallel over batch across 8 cores (4 batches/core).
Per-core dataflow (all matmuls fp32r unless noted):
  - host pre-transposes x -> xT [dim, n] per batch, pre-scales W_q by s,
    and pre-builds G[d, c] = rel_table[1024 - c, d] (bf16, duplicated on
    128 partitions) so the rel-pos product is a plain matmul.
  - qkvT projection: qT,kT in [channel, token] layout; v in [token, channel].
  - scores^T ... scores stay [i, j]; softmax along free dim j.
  - rel-pos: per i-tile, band B = qT_tile^T @ G_window  [128, 640] (bf16),
    bounced through DRAM and read back with an overlapping-stride AP
    (row stride 639 on a 640-wide buffer) which realizes
    pos[p, j] = B[p, 127 - p + j]; then accumulated into the dots PSUM
    with an identity matmul.
  - exp on ScalarE with accum_out giving row sums for free; normalize with
    per-partition tensor_scalar; transpose attn via xbar DMA; attn^T @ v
    gives out^T per head; W_out applied as lhsT natively; y^T written to
    DRAM and un-transposed on the host.
"""

import sys

for _p in ("/opt/trn_rl_repo",):
    if _p not in sys.path:
        sys.path.insert(0, _p)

import numpy as np
import ml_dtypes

import concourse.bass as bass
import concourse.mybir as mybir
import concourse.tile as tile
from concourse import bacc
from concourse import bass_utils
from concourse.masks import make_identity

F32 = mybir.dt.float32
F32R = mybir.dt.float32r
BF16 = mybir.dt.bfloat16

HEADS = 8
DH = 64
N = 512
DIM = 512
B_TOTAL = 32
NCORES = 8
BPC = B_TOTAL // NCORES  # batches per core
SCALE = DH ** -0.5
NT = N // 128  # 4 seq tiles
KT = DIM // 128  # 4 contraction tiles
GW = 1032  # padded G width (needs >= 1025)
BW = 640  # band width (needs >= 639)

AF = mybir.ActivationFunctionType


def r(ap):
    return ap.bitcast(F32R)


def build_program():
    nc = bacc.Bacc("TRN2", target_bir_lowering=False, debug=False)

    xT_d = nc.dram_tensor("xT", [BPC, DIM, N], F32R, kind="ExternalInput")
    w_d = nc.dram_tensor("w", [DIM, 3 * DIM], F32R, kind="ExternalInput")
    g_d = nc.dram_tensor("g", [128, GW], BF16, kind="ExternalInput")
    wout_d = nc.dram_tensor("wout", [DIM, DIM], BF16, kind="ExternalInput")
    bout_d = nc.dram_tensor("bout", [128, KT], F32, kind="ExternalInput")
    y_d = nc.dram_tensor("y", [BPC, DIM, N], F32, kind="ExternalOutput")

    from contextlib import ExitStack

    with ExitStack() as stack:
        tc = stack.enter_context(tile.TileContext(nc))
        ep = stack.enter_context
        const = ep(tc.tile_pool(name="const", bufs=1))
        xt_pool = ep(tc.tile_pool(name="xt", bufs=2))
        qk_pool = ep(tc.tile_pool(name="qk", bufs=2))
        qbf_pool = ep(tc.tile_pool(name="qbf", bufs=2))
        v_pool = ep(tc.tile_pool(name="vp", bufs=2))
        band_pool = ep(tc.tile_pool(name="band", bufs=4))
        pos_pool = ep(tc.tile_pool(name="pos", bufs=4))
        attn_pool = ep(tc.tile_pool(name="attn", bufs=8))
        attnn_pool = ep(tc.tile_pool(name="attnn", bufs=6))
        at_pool = ep(tc.tile_pool(name="at", bufs=2))
        outt_pool = ep(tc.tile_pool(name="outt", bufs=2))
        yt_pool = ep(tc.tile_pool(name="yt", bufs=4))
        small_pool = ep(tc.tile_pool(name="small", bufs=8))
        dband_pool = ep(tc.tile_pool(name="dbands", bufs=8, space="DRAM"))
        ps512 = ep(tc.tile_pool(name="ps512", bufs=2, space="PSUM"))
        psband = ep(tc.tile_pool(name="psband", bufs=2, space="PSUM"))
        psav = ep(tc.tile_pool(name="psav", bufs=2, space="PSUM"))
        if True:
            # ---- constants ----
            w_sb = []
            for kt in range(KT):
                t = const.tile([128, 3 * DIM], F32R, tag=f"w{kt}")
                nc.sync.dma_start(out=t, in_=w_d[kt * 128 : (kt + 1) * 128, :])
                w_sb.append(t)
            g_sb = const.tile([128, GW], BF16, tag="g")
            nc.sync.dma_start(out=g_sb, in_=g_d[:, :])
            wout_sb = []
            for ct in range(KT):
                t = const.tile([128, DIM], BF16, tag=f"wo{ct}")
                nc.sync.dma_start(out=t, in_=wout_d[ct * 128 : (ct + 1) * 128, :])
                wout_sb.append(t)
            bout_sb = const.tile([128, KT], F32, tag="bout")
            nc.sync.dma_start(out=bout_sb, in_=bout_d[:, :])
            ident = const.tile([128, 128], BF16, tag="ident")
            make_identity(nc, ident)

            for b in range(BPC):
                # ---- load xT ----
                xt_sb = []
                for kt in range(KT):
                    t = xt_pool.tile([128, N], F32R, tag=f"xt{kt}")
                    nc.sync.dma_start(
                        out=t, in_=xT_d[b, kt * 128 : (kt + 1) * 128, :]
                    )
                    xt_sb.append(t)

                # ---- qkT projection: channels on partitions ----
                qk_sb = []  # 8 tiles: q heads 2ct,2ct+1 then k heads
                qbf_sb = []  # bf16 copies of q tiles
                for ct in range(8):
                    ps = ps512.tile([128, N], F32, tag="mm512")
                    for kt in range(KT):
                        nc.tensor.matmul(
                            ps,
                            w_sb[kt][:, ct * 128 : (ct + 1) * 128],
                            xt_sb[kt][:, :],
                            start=(kt == 0),
                            stop=(kt == KT - 1),
                        )
                    t = qk_pool.tile([128, N], F32R, tag=f"qk{ct}")
                    nc.scalar.activation(t, ps, AF.Copy)
                    qk_sb.append(t)
                    if ct < 4:
                        tb = qbf_pool.tile([128, N], BF16, tag=f"qbf{ct}")
                        nc.vector.tensor_copy(tb, ps)
                        qbf_sb.append(tb)

                # ---- v in token-major layout ----
                v_sb = []
                for tt in range(NT):
                    ps = ps512.tile([128, N], F32, tag="mm512")
                    for kt in range(KT):
                        nc.tensor.matmul(
                            ps,
                            xt_sb[kt][:, tt * 128 : (tt + 1) * 128],
                            w_sb[kt][:, 2 * DIM : 3 * DIM],
                            start=(kt == 0),
                            stop=(kt == KT - 1),
                        )
                    t = v_pool.tile([128, DIM], BF16, tag=f"v{tt}")
                    nc.vector.tensor_copy(t, ps)
                    v_sb.append(t)

                # ---- per-batch output accumulator (attn out, channel-major) ----
                outt_sb = [
                    outt_pool.tile([128, N], BF16, tag=f"outt{ct}", name=f"outt{b}_{ct}")
                    for ct in range(KT)
                ]

                for h in range(HEADS):
                    hp = (h % 2) * 64
                    qT = qk_sb[h // 2][hp : hp + 64, :]
                    kTt = qk_sb[4 + h // 2][hp : hp + 64, :]
                    qbf = qbf_sb[h // 2][hp : hp + 64, :]

                    dps = []
                    sums = small_pool.tile([128, NT], F32, tag="sums")
                    for it in range(NT):
                        i0 = it * 128
                        # scores tile [128 i, 512 j]
                        dp = ps512.tile([128, N], F32, tag="mm512")
                        nc.tensor.matmul(
                            dp,
                            qT[:, i0 : i0 + 128],
                            kTt[:, :],
                            start=True,
                            stop=False,
                        )
                        dps.append(dp)

                        # rel-pos band [128, 640]
                        c_lo = 385 - i0
                        bp = psband.tile([128, BW], F32, tag="band")
                        nc.tensor.matmul(
                            bp[:, 0:512],
                            qbf[:, i0 : i0 + 128],
                            g_sb[hp : hp + 64, c_lo : c_lo + 512],
                            start=True,
                            stop=True,
                        )
                        nc.tensor.matmul(
                            bp[:, 512:BW],
                            qbf[:, i0 : i0 + 128],
                            g_sb[hp : hp + 64, c_lo + 512 : c_lo + BW],
                            start=True,
                            stop=True,
                        )
                        band_sb = band_pool.tile([128, BW], BF16, tag="band_sb")
                        nc.vector.tensor_copy(band_sb, bp)
                        dband = dband_pool.tile([128, BW], BF16, tag="dband")
                        nc.sync.dma_start(out=dband[:, :], in_=band_sb)
                        pos_sb = pos_pool.tile([128, N], BF16, tag="pos")
                        skew = bass.AP(
                            tensor=dband.tensor,
                            offset=dband.offset + 127,
                            ap=[[BW - 1, 128], [1, 512]],
                        )
                        nc.sync.dma_start(out=pos_sb, in_=skew)
                        # accumulate pos into scores psum
                        nc.tensor.matmul(dp, ident, pos_sb, start=False, stop=True)

                        at = attn_pool.tile([128, N], BF16, tag="attn")
                        nc.scalar.activation(
                            at, dp, AF.Exp, accum_out=sums[:, it : it + 1]
                        )
                        dps[it] = at  # keep sbuf exp tile

                    inv = small_pool.tile([128, NT], F32, tag="inv")
                    nc.vector.reciprocal(inv, sums)

                    atn = []
                    for it in range(NT):
                        t = attnn_pool.tile([128, N], BF16, tag="attnn")
                        nc.vector.tensor_scalar_mul(t, dps[it], inv[:, it : it + 1])
                        atn.append(t)

                    # transpose attn -> attnT tiles [128 j, 512 i]
                    at_sb = [
                        at_pool.tile([128, N], BF16, tag=f"at{jt}", name=f"at{b}_{h}_{jt}")
                        for jt in range(NT)
                    ]
                    for it in range(NT):
                        for jt in range(NT):
                            nc.sync.dma_start_transpose(
                                at_sb[jt][:, it * 128 : (it + 1) * 128],
                                atn[it][:, jt * 128 : (jt + 1) * 128],
                            )

                    # attn^T @ v -> out^T [64, 512] for this head
                    av = psav.tile([64, N], F32, tag="av")
                    for jt in range(NT):
                        nc.tensor.matmul(
                            av,
                            v_sb[jt][:, h * DH : (h + 1) * DH],
                            at_sb[jt][:, :],
                            start=(jt == 0),
                            stop=(jt == NT - 1),
                        )
                    nc.scalar.activation(
                        outt_sb[h // 2][hp : hp + 64, :], av, AF.Copy
                    )

                # ---- output projection: yT = W_out^T-contraction, [m, t] ----
                for mt in range(KT):
                    ps = ps512.tile([128, N], F32, tag="mm512")
                    for ct in range(KT):
                        nc.tensor.matmul(
                            ps,
                            wout_sb[ct][:, mt * 128 : (mt + 1) * 128],
                            outt_sb[ct][:, :],
                            start=(ct == 0),
                            stop=(ct == KT - 1),
                        )
                    yt = yt_pool.tile([128, N], F32, tag="yt")
                    nc.vector.tensor_scalar_add(yt, ps, bout_sb[:, mt : mt + 1])
                    nc.sync.dma_start(
                        out=y_d[b, mt * 128 : (mt + 1) * 128, :], in_=yt
                    )

    nc.finalize()
    return nc


_CACHE = {}


def _get_program():
    if "nc" not in _CACHE:
        _CACHE["nc"] = build_program()
    return _CACHE["nc"]


def _prep_inputs(x, W_qkv, rel_table, W_out, b_out):
    x = np.asarray(x, np.float32)
    W_qkv = np.asarray(W_qkv, np.float32)
    rel_table = np.asarray(rel_table, np.float32)
    W_out = np.asarray(W_out, np.float32)
    b_out = np.asarray(b_out, np.float32)

    w = W_qkv.copy()
    w[:, :DIM] *= SCALE  # fold softmax scale into q projection

    # G[d, c] = rel_table[1024 - c, d], padded to GW cols, rows duplicated
    g = np.zeros((128, GW), np.float32)
    g[:64, : 2 * N + 1] = rel_table[::-1].T
    g[64:128, :] = g[:64, :]
    g = g.astype(ml_dtypes.bfloat16)

    wout = W_out.astype(ml_dtypes.bfloat16)
    bout = b_out.reshape(KT, 128).T.copy()  # [128, KT]

    in_maps = []
    for c in range(NCORES):
        xs = x[c * BPC : (c + 1) * BPC]  # [BPC, n, dim]
        xT = np.ascontiguousarray(xs.transpose(0, 2, 1))
        in_maps.append(
            {"xT": xT, "w": w, "g": g, "wout": wout, "bout": bout}
        )
    return in_maps


def _run(inputs, trace=False):
    nc = _get_program()
    in_maps = _prep_inputs(**inputs)
    res = bass_utils.run_bass_kernel_spmd(
        nc, in_maps, core_ids=list(range(NCORES)), trace=trace
    )
    outs = [r["y"] for r in res.results]  # each [BPC, DIM(m), N(t)]
    y = np.concatenate(outs, axis=0)  # [32, m, t]
    y = np.ascontiguousarray(y.transpose(0, 2, 1))  # [32, t, m]
    return y, res


def kernel(**inputs):
    y, _ = _run(inputs, trace=False)
    return y
